# revision 1
# baseline (speedup 1.0000x reference)
"""DeepGCN (gnn_message_passing) Trainium2 Bass kernel, 8-way node-sharded SPMD.

Strategy (per core, nodes sharded 8 ways):
- Activations kept transposed hT [128 feats, RPAD rows] in SBUF, fp16
  (fp16 PE matmuls run at 1 cycle/row vs fp32's 4).
- Dense y = h@W + b: PE matmuls lhsT=hT-tile rhs=W (+rank-1 ones-matmul bias)
  -> row-major fp16 y tiles -> DMA to DRAM ag_in[layer][half] -> TWO
  AllGathers per layer (one per node-half) -> tables[layer][half]
  [TBL=HALF*8, W] (Shared).  Node-halves keep gather indices < 32768 (int16);
  the final layer pads C=64 -> W=128 so table rows stay 256B.
- Cross-layer pipelining: the next layer's dense blocks are emitted inside
  the previous spmm's superblock loop, and each half's AllGather triggers as
  soon as the dense blocks covering its rows are emitted (superblock 3 / 7),
  so collectives overlap the previous layer's gather/compute tail.
- spmm out[r] = sum_e val[e] * y[col[e]]: dma_gather 128 edges/tile into
  partitions (8-tile chunks = the 64-descriptor single_packet ceiling,
  round-robin over 4 SWDGE queues), selector SEL[e,r,t] =
  (iota==rowrel)*val batched per
  (superblock, half) in TWO DVE tensor_tensor ops with an [e, r, t] layout
  whose packed last dim enables the DVE 2x_1p fast mode, PE matmul
  g.T @ SEL accumulated in PSUM per 128-row block -> transposed result
  updates hT (relu/residual on the ACT engine).
- Final spmm uses lhsT=SEL, rhs=g -> row-major [rows, C] -> output shard.

Edges are preprocessed on host (numpy): sorted by destination row-block, split
per block into the two table halves, padded to a fixed number of 128-edge
tiles per (block, half) so one static program serves all 8 cores.
"""

import numpy as np

import concourse.bacc as bacc
import concourse.bass as bass
import concourse.mybir as mybir
import concourse.tile as tile
from concourse import library_config
from concourse.bass_utils import run_bass_kernel_spmd

NCORES = 8
P = 128


class Cfg:
    def __init__(self, N=40000, E=640000, DIN=256, H=128, C=64, L=2, SBB=5,
                 tbl_fp16=True):
        assert N % NCORES == 0
        self.N, self.E, self.DIN, self.H, self.C, self.L = N, E, DIN, H, C, L
        self.NSH = N // NCORES                    # rows per core
        self.NBLK = -(-self.NSH // P)             # 128-row blocks per core
        self.RPAD = self.NBLK * P
        assert self.NSH % 2 == 0
        self.HALF = self.NSH // 2                 # rows per table half per core
        self.TBL = self.HALF * NCORES             # rows per gather table
        assert self.TBL < 32768, "gather indices must fit int16"
        assert self.NBLK % SBB == 0
        self.SBB = SBB                            # blocks per superblock
        self.NSB = self.NBLK // SBB
        self.tbl_fp16 = tbl_fp16
        self.tdt = mybir.dt.float16 if tbl_fp16 else mybir.dt.float32
        self.tnp = np.float16 if tbl_fp16 else np.float32


CFG_FULL = Cfg()


# ---------------------------------------------------------------- host side


def _pack_idx(idx_flat):
    """[n] int16 -> [128, n//16]: slot i -> partition i%16, col i//16, x8 replicated."""
    n = idx_flat.shape[-1]
    t = idx_flat.reshape(*idx_flat.shape[:-1], n // 16, 16)
    t = np.swapaxes(t, -1, -2)                    # [..., 16, n//16]
    return np.tile(t, (1,) * (t.ndim - 2) + (8, 1)).astype(np.int16)


def _pack_pt(a_flat):
    """[n] -> [128, n//128]: slot i -> [i%128, i//128]."""
    n = a_flat.shape[-1]
    t = a_flat.reshape(*a_flat.shape[:-1], n // 128, 128)
    return np.swapaxes(t, -1, -2).copy()


def preprocess(cfg, x, edge_row, edge_col, edge_val):
    """Shard x, build per-core gather/selector metadata. Returns (per_core, TPB)."""
    er = np.asarray(edge_row).astype(np.int64)
    ec = np.asarray(edge_col).astype(np.int64)
    ev = np.asarray(edge_val).astype(np.float32)

    owner = er // cfg.NSH
    row_loc = er % cfg.NSH
    blk = row_loc // P                            # block within core
    rel = (row_loc % P).astype(np.float32)
    c_owner = ec // cfg.NSH
    c_loc = ec % cfg.NSH
    half = (c_loc >= cfg.HALF).astype(np.int64)
    tbl_idx = (c_owner * cfg.HALF + c_loc - half * cfg.HALF).astype(np.int64)

    cores = []
    max_cnt = 0
    for r in range(NCORES):
        m = owner == r
        cores.append((blk[m], half[m], tbl_idx[m], ev[m], rel[m]))
        key = blk[m] * 2 + half[m]
        cnt = np.bincount(key, minlength=cfg.NBLK * 2)
        max_cnt = max(max_cnt, int(cnt.max()))
    TPB = max(1, -(-max_cnt // P))                # tiles per (block, half)
    NIDX = cfg.SBB * TPB * P                      # gather-call size

    per_core = []
    for r in range(NCORES):
        b, h, ti, v, rl = cores[r]
        key = b * 2 + h
        # secondary sort by table index: monotone gather addresses within each
        # (block, half) group give far better HBM row locality
        order = np.argsort(key * 32768 + ti, kind="stable")
        b, h, ti, v, rl = b[order], h[order], ti[order], v[order], rl[order]
        cnt = np.bincount(key[order], minlength=cfg.NBLK * 2)
        # slot of edge j within its (b,h) group
        within = np.arange(len(b)) - np.repeat(
            np.concatenate([[0], np.cumsum(cnt)[:-1]]), cnt)
        # flat slot in [h, s, NIDX] layout
        s = b // cfg.SBB
        bb = b % cfg.SBB
        slot = bb * TPB * P + within
        idx_arr = np.zeros((2, cfg.NSB, NIDX), np.int16)
        val_arr = np.zeros((2, cfg.NSB, NIDX), np.float32)
        row_arr = np.zeros((2, cfg.NSB, NIDX), np.float32)
        idx_arr[h, s, slot] = ti.astype(np.int16)
        val_arr[h, s, slot] = v
        row_arr[h, s, slot] = rl

        xT = np.zeros((cfg.DIN, cfg.RPAD), cfg.tnp)
        xT[:, : cfg.NSH] = np.asarray(x[r * cfg.NSH:(r + 1) * cfg.NSH]).T
        per_core.append(dict(
            xT=np.ascontiguousarray(xT),
            idx=_pack_idx(idx_arr),                       # [2,NSB,128,NIDX//16]
            val=_pack_pt(val_arr).astype(cfg.tnp),        # [2,NSB,128,SBB*TPB]
            rowrel=_pack_pt(row_arr).astype(cfg.tnp),
        ))
    return per_core, TPB


# -------------------------------------------------------------- device side


def build_program(cfg, TPB, dt_val, no_cc=False, abl=(), reps=1, nq=4,
                  bufs=(4, 3, 2, 6, 4), qch=1, ch=8):
    abl = frozenset(abl)
    B_META, B_G, B_SEL, B_PSUM, B_Y = bufs
    H, C, DIN, L = cfg.H, cfg.C, cfg.DIN, cfg.L
    NIDX = cfg.SBB * TPB * P
    NPT = cfg.SBB * TPB
    fdims = [H] * (L + 1) + [C]                   # dense output feature dims
    # table slot widths: final layer pads C=64 -> 128 so rows stay 256B (fp16)
    # and the spmm matmuls run at 1 cycle/row instead of fp32's 4
    wdims = [H] * (L + 1) + [P]
    tdts = [cfg.tdt] * (L + 2)

    nc = bacc.Bacc("TRN2", target_bir_lowering=False, debug=False,
                   num_devices=NCORES, num_swdge_queues=nq)
    f32 = mybir.dt.float32

    adt = cfg.tdt                                 # activation/weight dtype
    xT_d = nc.dram_tensor("xT", [DIN, cfg.RPAD], adt, kind="ExternalInput")
    w1_d = nc.dram_tensor("w1", [DIN, H], adt, kind="ExternalInput")
    b1_d = nc.dram_tensor("b1", [1, H], f32, kind="ExternalInput")
    wm_d = nc.dram_tensor("wm", [L, H, H], adt, kind="ExternalInput")
    bm_d = nc.dram_tensor("bm", [L, 1, H], f32, kind="ExternalInput")
    w2_d = nc.dram_tensor("w2", [H, C], adt, kind="ExternalInput")
    b2_d = nc.dram_tensor("b2", [1, C], f32, kind="ExternalInput")
    iota_d = nc.dram_tensor("iota", [P, P], cfg.tdt, kind="ExternalInput")
    idx_d = nc.dram_tensor("idx", [2, cfg.NSB, P, NIDX // 16], mybir.dt.int16,
                           kind="ExternalInput")
    val_d = nc.dram_tensor("val", [2, cfg.NSB, P, NPT], cfg.tdt,
                           kind="ExternalInput")
    row_d = nc.dram_tensor("rowrel", [2, cfg.NSB, P, NPT], cfg.tdt,
                           kind="ExternalInput")
    out_d = nc.dram_tensor("out", [cfg.NSH, C], f32, kind="ExternalOutput")

    # per-half tensors: AllGather of half h only depends on dense blocks
    # covering rows [h*HALF, (h+1)*HALF), so AG(h=0) can run while the rest of
    # the pipeline (dense h=1, previous layer's spmm tail) still executes, and
    # spmm's h=0 gathers wait only on AG(h=0).
    ag_in = [[nc.dram_tensor(f"ag_in{l}_{h}", [cfg.HALF, wdims[l]], tdts[l])
              for h in (0, 1)] for l in range(L + 2)]
    tables = [[nc.dram_tensor(f"table{l}_{h}", [cfg.TBL, wdims[l]], tdts[l],
                              addr_space="Shared")
               for h in (0, 1)] for l in range(L + 2)]

    with tile.TileContext(nc) as tc:
        import contextlib
        with contextlib.ExitStack() as ctx:
            const = ctx.enter_context(tc.tile_pool(name="const", bufs=1))
            htp = ctx.enter_context(tc.tile_pool(name="ht", bufs=1))
            psum = ctx.enter_context(tc.tile_pool(name="psum", bufs=B_PSUM, space="PSUM"))
            meta = ctx.enter_context(tc.tile_pool(name="meta", bufs=B_META))
            gpool = ctx.enter_context(tc.tile_pool(name="g", bufs=B_G))
            selp = ctx.enter_context(tc.tile_pool(name="sel", bufs=B_SEL))
            yp = ctx.enter_context(tc.tile_pool(name="y", bufs=B_Y))

            nc.gpsimd.load_library(library_config.mlp)

            # ---- constants
            nkt = DIN // P                       # k-tiles for layer-1 dense
            w1_sb = [const.tile([P, H], adt, name=f"w1sb{k}")
                     for k in range(nkt)]
            for k in range(nkt):
                nc.sync.dma_start(w1_sb[k][:], w1_d[k * P:(k + 1) * P, :])
            b1_sb = const.tile([1, H], f32)
            nc.sync.dma_start(b1_sb[:], b1_d[:])
            wm_sb = [const.tile([P, H], adt, name=f"wmsb{i}")
                     for i in range(L)]
            bm_sb = [const.tile([1, H], f32, name=f"bmsb{i}")
                     for i in range(L)]
            for i in range(L):
                nc.sync.dma_start(wm_sb[i][:], wm_d[i])
                nc.sync.dma_start(bm_sb[i][:], bm_d[i])
            w2_sb = const.tile([P, C], adt)
            nc.sync.dma_start(w2_sb[:], w2_d[:])
            b2_sb = const.tile([1, C], f32)
            nc.sync.dma_start(b2_sb[:], b2_d[:])
            iota_sb = const.tile([P, P], cfg.tdt)
            nc.sync.dma_start(iota_sb[:], iota_d[:])
            ones_sb = const.tile([1, P], f32)
            nc.vector.memset(ones_sb[:], 1.0)
            # iota expanded over the tile dim with a packed last axis: the
            # selector build's operands then all have stride-1 last dims,
            # which is what DVE 2x_1p mode requires
            iota_x = const.tile([P, P, NPT], cfg.tdt)
            nc.vector.tensor_copy(
                out=iota_x[:],
                in_=iota_sb[:][:, :, None].to_broadcast([P, P, NPT]))

            ht = htp.tile([P, cfg.RPAD], adt)

            def dense(lhs_tiles, rhs_sb, bias_sb, F, l, odt, blocks):
                """y[m-block] = lhsT.T @ rhs + bias -> ag_in[l] (row-half split)."""
                for m in blocks:
                    ps = psum.tile([P, F], f32, tag="ps")
                    for k, lt in enumerate(lhs_tiles):
                        nc.tensor.matmul(
                            out=ps[:], lhsT=lt[:, m * P:(m + 1) * P], rhs=rhs_sb[k][:],
                            start=(k == 0), stop=False)
                    nc.tensor.matmul(out=ps[:], lhsT=ones_sb[:], rhs=bias_sb[:],
                                     start=False, stop=True)
                    ysb = yp.tile([P, F], odt, tag="ysb")
                    nc.scalar.activation(
                        out=ysb[:], in_=ps[:],
                        func=mybir.ActivationFunctionType.Copy)
                    r0 = m * P
                    r1 = min(cfg.NSH, r0 + P)
                    for h in (0, 1):
                        lo = max(r0, h * cfg.HALF)
                        hi = min(r1, (h + 1) * cfg.HALF)
                        if lo < hi:
                            nc.sync.dma_start(
                                out=ag_in[l][h][lo - h * cfg.HALF:
                                                hi - h * cfg.HALF, :F],
                                in_=ysb[lo - r0:hi - r0, :])

            def allgather(l, h):
                if no_cc:
                    # timeline-profiling stand-in: local copy, same deps
                    nc.sync.dma_start(out=tables[l][h][0:cfg.HALF, :],
                                      in_=ag_in[l][h][:])
                    return
                if "agact" in abl:
                    # issue the collective trigger from the SP/DMA queue so
                    # it never stalls the gather stream on gpsimd
                    nc.has_collectives = True
                    nc.sync.add_instruction(
                        mybir.InstCollectiveCompute(
                            name=f"I-{nc.next_id()}",
                            kind="AllGather",
                            op=mybir.AluOpType.bypass,
                            replica_groups=[list(range(NCORES))],
                            ins=[nc.sync.lower_ap(ag_in[l][h][:])],
                            outs=[nc.sync.lower_ap(tables[l][h][:])],
                            unique_tensors="No",
                            cc_dim="Partition",
                        ))
                    return
                nc.gpsimd.collective_compute(
                    "AllGather", mybir.AluOpType.bypass,
                    replica_groups=[list(range(NCORES))],
                    ins=[ag_in[l][h][:]], outs=[tables[l][h][:]])

            def spmm(l, after_sb=None):
                """tables[l] -> block outputs; updates ht (l<=L) or out (final).

                after_sb(s) is called after each superblock's outputs are
                emitted — used to interleave the next layer's dense blocks and
                AllGather triggers into the spmm stream for cross-layer overlap.
                """
                final = l == L + 1
                F = fdims[l]
                W = wdims[l]
                sdt = tdts[l]
                io_t = iota_sb
                v_d, r_d = val_d, row_d
                mdt = cfg.tdt
                for s in range(cfg.NSB):
                    g = []
                    vt = []
                    rt = []
                    for h in (0, 1):
                        it = meta.tile([P, NIDX // 16], mybir.dt.int16,
                                       tag=f"it{h}")
                        nc.sync.dma_start(it[:], idx_d[h, s])
                        if "zeroidx" in abl:
                            nc.vector.memset(it[:], 0)
                        v = meta.tile([P, NPT], mdt, tag=f"vt{h}")
                        nc.sync.dma_start(v[:], v_d[h, s])
                        rr = meta.tile([P, NPT], mdt, tag=f"rt{h}")
                        nc.sync.dma_start(rr[:], r_d[h, s])
                        gt = gpool.tile([P, NPT, W], sdt, tag=f"g{h}")
                        # chunk calls to <=56 descs/lane: single_packet=True
                        # coalesces each lane's stream into ONE packet and the
                        # HW packet ceiling is 64 descriptors; 7 tiles = 56
                        # keeps margin (8 tiles = 64 verified but no faster)
                        # single_packet coalesces each lane's stream into ONE
                        # packet (HW ceiling 64 descriptors = 8 tiles); larger
                        # chunks need single_packet=False but amortize the
                        # ~1us fixed SWDGE cost over more descriptors
                        CH = ch
                        for c0 in range(0, NPT, CH):
                            c1 = min(NPT, c0 + CH)
                            if "nogather" in abl:
                                break
                            nc.gpsimd.dma_gather(
                                gt[:, c0:c1, :],
                                tables[l][h][:, :],
                                it[:, c0 * 8:c1 * 8],
                                (c1 - c0) * P, (c1 - c0) * P, W,
                                single_packet=(CH <= 8),
                                queue_num=(s * 2 + h + (c0 // CH) * qch) % nq)
                        g.append(gt)
                        vt.append(v)
                        rt.append(rr)
                    # batched selector build, [e, r, t] layout: one is_equal
                    # + one mult per (superblock, half).  All operands keep a
                    # packed (stride-1) last dim -> DVE 2x_1p fast mode.
                    sels = []
                    for h in (0, 1):
                        sel_a = selp.tile([P, P, NPT], mdt, tag=f"sel{h}")
                        nc.vector.tensor_tensor(
                            out=sel_a[:],
                            in0=iota_x[:],
                            in1=rt[h][:, None, :].to_broadcast([P, P, NPT]),
                            op=mybir.AluOpType.is_equal)
                        if "nosel" in abl:
                            sels.append(sel_a)
                            continue
                        nc.vector.tensor_tensor(
                            out=sel_a[:], in0=sel_a[:],
                            in1=vt[h][:, None, :].to_broadcast([P, P, NPT]),
                            op=mybir.AluOpType.mult)
                        sels.append(sel_a)
                    for bb in range(cfg.SBB):
                        b = s * cfg.SBB + bb
                        if final:
                            ps = psum.tile([P, W], f32, tag="ps")
                        else:
                            ps = psum.tile([F, P], f32, tag="ps")
                        k = 0
                        for h in (0, 1):
                            if "nomm" in abl:
                                nc.vector.memset(ps[:], 0.0)
                                break
                            for t in range(TPB):
                                j = bb * TPB + t
                                sel = sels[h][:, :, j]
                                if final:
                                    nc.tensor.matmul(
                                        out=ps[:], lhsT=sel, rhs=g[h][:, j, :],
                                        start=(k == 0), stop=(k == 2 * TPB - 1))
                                else:
                                    nc.tensor.matmul(
                                        out=ps[:], lhsT=g[h][:, j, :], rhs=sel,
                                        start=(k == 0), stop=(k == 2 * TPB - 1))
                                k += 1
                        if final:
                            osb = yp.tile([P, F], f32, tag="osb")
                            nc.scalar.activation(
                                out=osb[:], in_=ps[:, :F],
                                func=mybir.ActivationFunctionType.Copy)
                            r0 = b * P
                            r1 = min(cfg.NSH, r0 + P)
                            if r0 < r1:
                                nc.sync.dma_start(out=out_d[r0:r1, :],
                                                  in_=osb[: r1 - r0, :])
                        elif l == 0:
                            nc.scalar.activation(
                                out=ht[:, b * P:(b + 1) * P], in_=ps[:],
                                func=mybir.ActivationFunctionType.Relu)
                        else:
                            tmp = yp.tile([P, P], adt, tag="tmp")
                            nc.scalar.activation(
                                out=tmp[:], in_=ps[:],
                                func=mybir.ActivationFunctionType.Relu,
                                scale=float(dt_val))
                            nc.vector.tensor_add(
                                out=ht[:, b * P:(b + 1) * P],
                                in0=ht[:, b * P:(b + 1) * P], in1=tmp[:])
                    if after_sb is not None:
                        after_sb(s)

            # block 19 straddles the HALF boundary: AG(h=0) needs blocks
            # 0..19 done, AG(h=1) needs 19..39.  Superblock 3 = blocks 15-19.
            sb_lo = (cfg.HALF // P) // cfg.SBB     # superblock containing blk 19
            dense_args = [None] * (L + 2)

            def chain(l_next):
                """after_sb hook: emit next layer's dense per superblock and
                trigger each table-half AllGather as soon as its rows exist."""
                def hook(s):
                    lhs, rhs, bias, F, odt = dense_args[l_next]
                    dense(lhs, rhs, bias, F, l_next, odt,
                          range(s * cfg.SBB, (s + 1) * cfg.SBB))
                    if s == sb_lo:
                        allgather(l_next, 0)
                    if s == cfg.NSB - 1:
                        allgather(l_next, 1)
                return hook

            for _rep in range(reps):
                with tc.tile_pool(name="xt", bufs=1) as xtp:
                    xt_sb = xtp.tile([P, nkt * cfg.RPAD], adt)
                    # chunked load: dense block 0 can start after 1/8 of the
                    # xT DMA instead of waiting for the whole 2.6MB
                    XCH = cfg.RPAD // 8
                    for c0 in range(0, cfg.RPAD, XCH):
                        for k in range(nkt):
                            nc.sync.dma_start(
                                xt_sb[:, k * cfg.RPAD + c0:
                                      k * cfg.RPAD + c0 + XCH],
                                xT_d[k * P:(k + 1) * P, c0:c0 + XCH])
                    xts = [xt_sb[:, k * cfg.RPAD:(k + 1) * cfg.RPAD]
                           for k in range(nkt)]
                    dense(xts, w1_sb, b1_sb, H, 0, tdts[0],
                          range(0, (sb_lo + 1) * cfg.SBB))
                    allgather(0, 0)
                    dense(xts, w1_sb, b1_sb, H, 0, tdts[0],
                          range((sb_lo + 1) * cfg.SBB, cfg.NBLK))
                    allgather(0, 1)
                for i in range(L):
                    dense_args[i + 1] = ([ht], [wm_sb[i]], bm_sb[i], H,
                                         tdts[i + 1])
                    spmm(i, after_sb=chain(i + 1))
                dense_args[L + 1] = ([ht], [w2_sb], b2_sb, C, tdts[L + 1])
                spmm(L, after_sb=chain(L + 1))
                spmm(L + 1)

    nc.compile()
    return nc


# ------------------------------------------------------------------ driver

_CACHE = {}


def _get_program(cfg, TPB, dt_val):
    key = (cfg.N, cfg.E, cfg.tbl_fp16, TPB, float(dt_val))
    if key not in _CACHE:
        _CACHE[key] = build_program(cfg, TPB, dt_val)
    return _CACHE[key]


def prepare(cfg, inputs):
    """Preprocess inputs and build (cached) program. Returns (nc, in_maps)."""
    x = np.asarray(inputs["x"], np.float32)
    per_core, TPB = preprocess(cfg, x, inputs["edge_row"], inputs["edge_col"],
                               inputs["edge_val"])
    dt_val = float(np.asarray(inputs["time_step"]))
    nc = _get_program(cfg, TPB, dt_val)

    iota32 = np.tile(np.arange(P, dtype=np.float32), (P, 1))
    shared = dict(
        w1=np.asarray(inputs["w1"], cfg.tnp),
        b1=np.asarray(inputs["b1"], np.float32).reshape(1, cfg.H),
        wm=np.asarray(inputs["wm"], cfg.tnp),
        bm=np.asarray(inputs["bm"], np.float32).reshape(cfg.L, 1, cfg.H),
        w2=np.asarray(inputs["w2"], cfg.tnp),
        b2=np.asarray(inputs["b2"], np.float32).reshape(1, cfg.C),
        iota=iota32.astype(cfg.tnp),
    )
    in_maps = [{**shared, **pc} for pc in per_core]
    return nc, in_maps


def run(cfg, inputs):
    nc, in_maps = prepare(cfg, inputs)
    res = run_bass_kernel_spmd(nc, in_maps, list(range(NCORES)))
    out = np.concatenate([res.results[r]["out"] for r in range(NCORES)], axis=0)
    return out.astype(np.float32)


def kernel(**inputs) -> np.ndarray:
    return run(CFG_FULL, inputs)


# ---------------------------------------------------- timing helper (test use)


def make_timed_runner(nc, in_maps):
    """Build a reusable jitted runner (no donation, device-resident operands).

    Mirrors bass2jax.run_bass_via_pjrt's multi-core path but keeps the jitted
    callable and device arrays so repeated calls measure dispatch+exec only.
    Returns (call_fn, out_unpack_fn).
    """
    import jax
    from jax.sharding import Mesh, PartitionSpec
    from jax.experimental.shard_map import shard_map
    from concourse import bass2jax
    from concourse.bass2jax import _bass_exec_p, partition_id_tensor

    bass2jax.install_neuronx_cc_hook()
    n_cores = len(in_maps)
    partition_name = nc.partition_id_tensor.name if nc.partition_id_tensor else None
    in_names, out_names, out_avals, zero_outs = [], [], [], []
    for alloc in nc.m.functions[0].allocations:
        if not isinstance(alloc, mybir.MemoryLocationSet):
            continue
        name = alloc.memorylocations[0].name
        if alloc.kind == "ExternalInput":
            if name != partition_name:
                in_names.append(name)
        elif alloc.kind == "ExternalOutput":
            out_names.append(name)
            out_avals.append(jax.core.ShapedArray(
                tuple(alloc.tensor_shape), mybir.dt.np(alloc.dtype)))
            zero_outs.append(np.zeros(tuple(alloc.tensor_shape),
                                      mybir.dt.np(alloc.dtype)))
    n_params = len(in_names)
    all_names = in_names + out_names
    if partition_name is not None:
        all_names.append(partition_name)

    def _body(*args):
        operands = list(args)
        if partition_name is not None:
            operands.append(partition_id_tensor())
        return tuple(_bass_exec_p.bind(
            *operands,
            out_avals=tuple(out_avals),
            in_names=tuple(all_names),
            out_names=tuple(out_names),
            lowering_input_output_aliases=(),
            sim_require_finite=True,
            sim_require_nnan=True,
            nc=nc,
        ))

    devices = jax.devices()[:n_cores]
    mesh = Mesh(np.asarray(devices), ("core",))
    spec_in = (PartitionSpec("core"),) * (n_params + len(out_names))
    spec_out = (PartitionSpec("core"),) * len(out_names)
    fn = jax.jit(shard_map(_body, mesh=mesh, in_specs=spec_in,
                           out_specs=spec_out, check_rep=False),
                 keep_unused=True)

    sharding = jax.sharding.NamedSharding(mesh, PartitionSpec("core"))
    dev_args = []
    for i, name in enumerate(in_names):
        cat = np.concatenate([np.asarray(m[name]) for m in in_maps], axis=0)
        dev_args.append(jax.device_put(cat, sharding))
    for z in zero_outs:
        cat = np.zeros((n_cores * z.shape[0], *z.shape[1:]), z.dtype)
        dev_args.append(jax.device_put(cat, sharding))

    def call():
        outs = fn(*dev_args)
        jax.block_until_ready(outs)
        return outs

    def unpack(outs):
        return [
            {name: np.asarray(outs[i]).reshape(n_cores, *out_avals[i].shape)[c]
             for i, name in enumerate(out_names)}
            for c in range(n_cores)
        ]

    return call, unpack

